# revision 66
# baseline (speedup 1.0000x reference)
"""AttentionJacobian kernel for 8 TRN2 NeuronCores.

J[b,q] = scale * ( V^T diag(a_q) K  -  o_q w_q^T ),  a = softmax(Q K^T scale)

Data-parallel over batch: 16 batches -> 2 per core. Per batch on-device:
  scoresT chunks (n x q) = KT_c^T @ QT        (bf16 matmuls)
  E = exp(scale * scoresT)                    (ScalarE, bf16; no max-sub
                                               needed: |scale*s| small)
  Z = ones^T @ E, rzb = SCALE/Z bcast         (PE + DVE reciprocal)
  AT = E * rzb                                (Pool, bf16, SCALE folded in)
  per 16-q block: ADup = AT pairs [p,c,j,2]   (ScalarE copy)
  SK units (4 chunks x 16 q): sk = K (*) a    as ONE DVE tensor_tensor in
    2x_1P packed mode -- both operands are step-1 bf16 reads (K pairs from
    a dedicated KX copy, a-pairs from ADup), 2 elem/lane/cycle.
    All SK on VectorE: Pool/ACT are 4x/3.5x slower per element and any
    GpSimd load trips the chip power duty-cycler (10.24us k-of-n windows),
    inflating every engine ~2.5x.
  term1 psum += V_c^T @ sk                    (bf16 matmuls, f32 psum,
                                               512-col groups, 8 banks)
  term2: rank-1 PE matmuls psum += (-o_q/SCALE) (x) w_q  (K=1, operands
    are [1,128] rows of SBUF-flattened w/negO at partition 0)
  evacuate psum -> bf16 jsb (ScalarE) -> HBM; host upcasts to f32.
Both batches' softmax phases are emitted before the block loops so the
Vector queue never stalls mid-stream; input DMAs are ordered QT/KT-pieces
first (they gate scores), then KV/KX bulk.
"""

import sys

for p in ("/opt/trn_rl_repo",):
    if p not in sys.path:
        sys.path.append(p)

import numpy as np
import ml_dtypes

import concourse.bass as bass
import concourse.bacc as bacc
import concourse.tile as tile
from concourse import mybir
from concourse.bass_utils import run_bass_kernel_spmd

N_CORES = 8
BATCH = 16
NQ = 64
SEQ = 4096
D = 128
BPC = BATCH // N_CORES        # batches per core = 2
C = SEQ // 128                # 32 contraction chunks
QG = 8                        # q per output group
NG = NQ // QG                 # 8 groups
SCALE = float(D) ** -0.5

F32 = mybir.dt.float32
BF16 = mybir.dt.bfloat16
AF = mybir.ActivationFunctionType
ALU = mybir.AluOpType

_CACHED = {}


def _build():
    nc = bacc.Bacc("TRN2", target_bir_lowering=False, debug=False,
                   num_devices=N_CORES)

    kvb = nc.dram_tensor("kvb", [BPC, C, 128, 256], BF16, kind="ExternalInput").ap()
    kt = nc.dram_tensor("kt", [BPC, 128, SEQ], BF16, kind="ExternalInput").ap()
    qt = nc.dram_tensor("qt", [BPC, 128, NQ], BF16, kind="ExternalInput").ap()
    out = nc.dram_tensor("out", [BPC, NQ, D, D], BF16, kind="ExternalOutput").ap()

    with tile.TileContext(nc) as tc:
        with (
            tc.tile_pool(name="const", bufs=1) as constp,
            tc.tile_pool(name="kv", bufs=2) as kvp,
            tc.tile_pool(name="ktp", bufs=2) as ktp,
            tc.tile_pool(name="qtp", bufs=2) as qtp,
            tc.tile_pool(name="ep", bufs=2) as ep,
            tc.tile_pool(name="atp", bufs=2) as atp,
            tc.tile_pool(name="rzp", bufs=2) as rzp,
            tc.tile_pool(name="owp", bufs=2) as owp,
            tc.tile_pool(name="flatp", bufs=1) as flatp,
            tc.tile_pool(name="skp", bufs=4) as skp,
            tc.tile_pool(name="jsbp", bufs=3) as jsbp,
            tc.tile_pool(name="psj", bufs=3, space="PSUM") as psjp,
            tc.tile_pool(name="pss", bufs=1, space="PSUM") as pssp,
            tc.tile_pool(name="psmall", bufs=1, space="PSUM") as psmp,
        ):
            onescol = constp.tile([128, 1], BF16)
            nc.vector.memset(onescol[:, :], 1.0)
            onesrowS = constp.tile([1, 128], F32)
            nc.vector.memset(onesrowS[:, :], SCALE)

            def phase(b):
                # DMA order: QT + KT pieces first (they gate the scores
                # matmuls and everything downstream), then KV/KX bulk.
                QT = qtp.tile([128, NQ], BF16, tag="qt")
                nc.sync.dma_start(QT[:, :], qt[b])
                KT = ktp.tile([128, SEQ], BF16, tag="kt")
                KTPC = 4  # KT DMA pieces
                for piece in range(KTPC):
                    w = SEQ // KTPC
                    nc.sync.dma_start(KT[:, piece * w:(piece + 1) * w],
                                      kt[b, :, piece * w:(piece + 1) * w])

                # --- softmax numerator: E = exp(scale * K Q^T), 2 chunks/exp
                E = ep.tile([128, C * NQ], BF16, tag="e")
                for cc in range(C // 2):
                    ps_s = pssp.tile([128, 2 * NQ], F32, tag="scores")
                    for ci in range(2):
                        c = 2 * cc + ci
                        nc.tensor.matmul(ps_s[:, ci * NQ:(ci + 1) * NQ],
                                         KT[:, c * 128:(c + 1) * 128],
                                         QT[:, :], start=True, stop=True)
                    nc.scalar.activation(E[:, 2 * cc * NQ:(2 * cc + 2) * NQ],
                                         ps_s[:, :], AF.Exp,
                                         bias=0.0, scale=SCALE)

                KV = kvp.tile([128, C * 256], BF16, tag="kv")
                nc.sync.dma_start(KV[:, :].rearrange("p (c j) -> p c j", j=256),
                                  kvb[b].rearrange("c n j -> n c j"))
                KX = kvp.tile([128, C * 128], BF16, tag="kx")
                nc.sync.dma_start(KX[:, :].rearrange("p (c j) -> p c j", j=128),
                                  kvb[b, :, :, 128:256].rearrange("c n j -> n c j"))

                # --- Z (1 x NQ) = ones^T E ; rzb = SCALE/Z bcast to 128 parts
                ps_z = psmp.tile([128, 128], F32, tag="small")
                for c in range(C):
                    nc.tensor.matmul(ps_z[0:1, 0:NQ], onescol[:, :],
                                     E[:, c * NQ:(c + 1) * NQ],
                                     start=(c == 0), stop=(c == C - 1))
                rz = rzp.tile([1, NQ], F32, tag="rz")
                nc.vector.reciprocal(rz[:, :], ps_z[0:1, 0:NQ])
                ps_rzb = psmp.tile([128, 128], F32, tag="small")
                nc.tensor.matmul(ps_rzb[:, 0:NQ], onesrowS[:, :], rz[:, :],
                                 start=True, stop=True)

                # --- AT = E * rzb  (chunk-major layout [c*NQ + q])
                rzbsb = rzp.tile([128, NQ], F32, tag="rzbsb")
                nc.scalar.copy(rzbsb[:, :], ps_rzb[:, 0:NQ])
                AT = atp.tile([128, C * NQ], BF16, tag="at")
                H = C // 2
                for hh in range(2):
                    nc.gpsimd.tensor_mul(
                        AT[:, hh * H * NQ:(hh + 1) * H * NQ]
                        .rearrange("p (c q) -> p c q", q=NQ),
                        E[:, hh * H * NQ:(hh + 1) * H * NQ]
                        .rearrange("p (c q) -> p c q", q=NQ),
                        rzbsb[:, :].unsqueeze(1).broadcast_to((128, H, NQ)),
                    )

                # --- w rows [q, k] and o rows [q, v]  (both q-partition)
                ps_ow = psmp.tile([128, 128], F32, tag="small")
                for c in range(C):
                    nc.tensor.matmul(ps_ow[0:NQ, 0:128],
                                     AT[:, c * NQ:(c + 1) * NQ],
                                     KV[:, c * 256 + 128:(c + 1) * 256],
                                     start=(c == 0), stop=(c == C - 1))
                wsb = owp.tile([NQ, 128], BF16, tag="wsb")
                nc.scalar.copy(wsb[:, :], ps_ow[0:NQ, 0:128])
                wflat = flatp.tile([1, NQ * 128], BF16, tag="wflat")
                nc.sync.dma_start(
                    wflat[:, :].rearrange("o (q k) -> o q k", k=128),
                    wsb[:, :])

                ps_o = psmp.tile([128, 128], F32, tag="small")
                for c in range(C):
                    nc.tensor.matmul(ps_o[0:NQ, 0:128],
                                     AT[:, c * NQ:(c + 1) * NQ],
                                     KV[:, c * 256:c * 256 + 128],
                                     start=(c == 0), stop=(c == C - 1))
                negO = owp.tile([NQ, 128], BF16, tag="nego")
                nc.scalar.activation(negO[:, :], ps_o[0:NQ, 0:128],
                                     AF.Copy, bias=0.0, scale=-1.0 / SCALE)
                negOflat = flatp.tile([1, NQ * 128], BF16, tag="negoflat")
                nc.sync.dma_start(
                    negOflat[:, :].rearrange("o (q v) -> o q v", v=128),
                    negO[:, :])
                return dict(KV=KV, KX=KX, AT=AT, wflat=wflat,
                            negOflat=negOflat)

            QB = 2 * QG  # 16 q per block

            adup_tiles = {}

            def build_adup(ctx, b, blk):
                # pair-duplicated AT for a block's 16 q: [p, c, j, 2]
                AT = ctx["AT"]
                ADup = atp.tile([128, C * QB * 2], BF16, tag="adup")
                H = C // 2
                for hh in range(2):
                    nc.scalar.copy(
                        ADup[:, hh * H * QB * 2:(hh + 1) * H * QB * 2]
                        .rearrange("p (c j two) -> p c j two", j=QB, two=2),
                        AT[:, hh * H * NQ:(hh + 1) * H * NQ]
                        .rearrange("p (c q) -> p c q", q=NQ)
                        [:, :, blk * QB:(blk + 1) * QB].unsqueeze(3)
                        .broadcast_to((128, H, QB, 2)),
                    )
                adup_tiles[(b, blk)] = ADup

            def blocks(b, ctx, blk, nxt):
                KV, KX = ctx["KV"], ctx["KX"]
                wflat, negOflat = ctx["wflat"], ctx["negOflat"]
                ADup = adup_tiles.pop((b, blk))
                ps_a = psjp.tile([128, QG * 128], F32, tag="j")
                ps_b = psjp.tile([128, QG * 128], F32, tag="j")
                CU = 4  # chunks per SK unit
                for cc in range(C // CU):
                    c0 = CU * cc
                    sk = skp.tile([128, CU * QB * 128], BF16, tag="sk")
                    # dup-packed 2x_1P: both operands step-1 bf16
                    nc.vector.tensor_mul(
                        sk[:, :].rearrange(
                            "p (ci j kp two) -> p ci j kp two",
                            j=QB, kp=64, two=2),
                        KX[:, :].rearrange(
                            "p (c kp two) -> p c kp two", kp=64, two=2)
                        [:, c0:c0 + CU].unsqueeze(2)
                        .broadcast_to((128, CU, QB, 64, 2)),
                        ADup[:, :].rearrange(
                            "p (c j two) -> p c j two", j=QB, two=2)
                        [:, c0:c0 + CU].unsqueeze(3)
                        .broadcast_to((128, CU, QB, 64, 2)),
                    )
                    for ci in range(CU):
                        c = c0 + ci
                        for h, ps in enumerate((ps_a, ps_a, ps_b, ps_b)):
                            nc.tensor.matmul(
                                ps[:, (h % 2) * 512:(h % 2) * 512 + 512],
                                KV[:, c * 256:c * 256 + 128],
                                sk[:, ci * 2048 + h * 512:
                                   ci * 2048 + (h + 1) * 512],
                                start=(c == 0), stop=False,
                                skip_group_check=True)
                    if cc == 2 and nxt is not None:
                        # prebuild next block's ADup now, ahead of the jsb
                        # copies on the in-order ScalarE queue
                        build_adup(*nxt)
                for half, ps in enumerate((ps_a, ps_b)):
                    jsb = jsbp.tile([128, QG * 128], BF16, tag="jsb")
                    q0 = blk * QB + half * QG
                    # evacuate each 512-col bank right after its rank-1s so
                    # copy/DMA of bank 0 overlaps bank 1's rank-1 matmuls
                    for bank in range(2):
                        for j in range(bank * 4, bank * 4 + 4):
                            q = q0 + j
                            nc.tensor.matmul(ps[:, j * 128:(j + 1) * 128],
                                             negOflat[:, q * 128:(q + 1) * 128],
                                             wflat[:, q * 128:(q + 1) * 128],
                                             start=False, stop=True,
                                             skip_group_check=True,
                                             tile_position=(0, 0))
                        nc.scalar.copy(jsb[:, bank * 512:(bank + 1) * 512],
                                       ps[:, bank * 512:(bank + 1) * 512])
                        nc.sync.dma_start(
                            out[b, q0 + bank * 4:q0 + bank * 4 + 4]
                            .rearrange("j v k -> v j k"),
                            jsb[:, bank * 512:(bank + 1) * 512]
                            .rearrange("v (j k) -> v j k", k=128),
                        )

            # interleave: emit batch 1's phase right after batch 0's first
            # block so its AT is ready when Vector drains batch 0's SK.
            ctx0 = phase(0)
            ctx1 = phase(1)
            seq = [(ctx0, 0, blk) for blk in range(NQ // QB)]
            seq += [(ctx1, 1, blk) for blk in range(NQ // QB)]
            build_adup(*seq[0])
            for i, (ctx, b, blk) in enumerate(seq):
                nxt = seq[i + 1] if i + 1 < len(seq) else None
                blocks(b, ctx, blk, nxt)

    nc.compile()
    return nc


def _get_nc():
    if "nc" not in _CACHED:
        _CACHED["nc"] = _build()
    return _CACHED["nc"]


def _prep_core_inputs(query, keys, values, i):
    s = slice(i * BPC, (i + 1) * BPC)
    K = np.ascontiguousarray(keys[s])     # (2, 4096, 128) f32
    V = np.ascontiguousarray(values[s])
    Q = np.ascontiguousarray(query[s])    # (2, 64, 128) f32
    kvb = np.empty((BPC, C, 128, 256), dtype=ml_dtypes.bfloat16)
    kvb[:, :, :, 0:128] = V.reshape(BPC, C, 128, 128)
    kvb[:, :, :, 128:256] = K.reshape(BPC, C, 128, 128)
    kt = np.ascontiguousarray(K.transpose(0, 2, 1)).astype(ml_dtypes.bfloat16)
    qt = np.ascontiguousarray(Q.transpose(0, 2, 1)).astype(ml_dtypes.bfloat16)
    return {"kvb": kvb, "kt": kt, "qt": qt}


def kernel(query, keys, values):
    query = np.asarray(query, dtype=np.float32)
    keys = np.asarray(keys, dtype=np.float32)
    values = np.asarray(values, dtype=np.float32)
    nc = _get_nc()
    in_maps = [_prep_core_inputs(query, keys, values, i) for i in range(N_CORES)]
    res = run_bass_kernel_spmd(nc, in_maps, core_ids=list(range(N_CORES)))
    return np.concatenate(
        [np.asarray(res.results[i]["out"]) for i in range(N_CORES)],
        axis=0).astype(np.float32)


# revision 73
# speedup vs baseline: 1.0185x; 1.0185x over previous
"""AttentionJacobian kernel for 8 TRN2 NeuronCores.

J[b,q] = scale * ( V^T diag(a_q) K  -  o_q w_q^T ),  a = softmax(Q K^T scale)

Data-parallel over batch: 16 batches -> 2 per core. Per batch on-device:
  scoresT chunks (n x q) = KT_c^T @ QT        (bf16 matmuls)
  E = exp(scale * scoresT)                    (ScalarE, bf16; no max-sub
                                               needed: |scale*s| small)
  Z = ones^T @ E, rzb = SCALE/Z bcast         (PE + DVE reciprocal)
  AT = E * rzb                                (Pool, bf16, SCALE folded in)
  per 16-q block: ADup = AT pairs [p,c,j,2]   (ScalarE copy)
  SK units (4 chunks x 16 q): sk = K (*) a    as ONE DVE tensor_tensor in
    2x_1P packed mode -- both operands are step-1 bf16 reads (K pairs from
    a dedicated KX copy, a-pairs from ADup), 2 elem/lane/cycle.
    All SK on VectorE: Pool/ACT are 4x/3.5x slower per element and any
    GpSimd load trips the chip power duty-cycler (10.24us k-of-n windows),
    inflating every engine ~2.5x.
  term1 psum += V_c^T @ sk                    (bf16 matmuls, f32 psum,
                                               512-col groups, 8 banks)
  term2: rank-1 PE matmuls psum += (-o_q/SCALE) (x) w_q  (K=1, operands
    are [1,128] rows of SBUF-flattened w/negO at partition 0)
  evacuate psum -> bf16 jsb (ScalarE) -> HBM; host upcasts to f32.
Both batches' softmax phases are emitted before the block loops so the
Vector queue never stalls mid-stream; input DMAs are ordered QT/KT-pieces
first (they gate scores), then KV/KX bulk.
"""

import sys

for p in ("/opt/trn_rl_repo",):
    if p not in sys.path:
        sys.path.append(p)

import numpy as np
import ml_dtypes

import concourse.bass as bass
import concourse.bacc as bacc
import concourse.tile as tile
from concourse import mybir
from concourse.bass_utils import run_bass_kernel_spmd

N_CORES = 8
BATCH = 16
NQ = 64
SEQ = 4096
D = 128
BPC = BATCH // N_CORES        # batches per core = 2
C = SEQ // 128                # 32 contraction chunks
QG = 8                        # q per output group
NG = NQ // QG                 # 8 groups
SCALE = float(D) ** -0.5

F32 = mybir.dt.float32
BF16 = mybir.dt.bfloat16
AF = mybir.ActivationFunctionType
ALU = mybir.AluOpType

_CACHED = {}


def _build():
    nc = bacc.Bacc("TRN2", target_bir_lowering=False, debug=False,
                   num_devices=N_CORES)

    kvb = nc.dram_tensor("kvb", [BPC, C, 128, 256], BF16, kind="ExternalInput").ap()
    kt = nc.dram_tensor("kt", [BPC, 128, SEQ], BF16, kind="ExternalInput").ap()
    qt = nc.dram_tensor("qt", [BPC, 128, NQ], BF16, kind="ExternalInput").ap()
    out = nc.dram_tensor("out", [BPC, NQ, D, D], BF16, kind="ExternalOutput").ap()

    with tile.TileContext(nc) as tc:
        with (
            tc.tile_pool(name="const", bufs=1) as constp,
            tc.tile_pool(name="kv", bufs=2) as kvp,
            tc.tile_pool(name="ktp", bufs=2) as ktp,
            tc.tile_pool(name="qtp", bufs=2) as qtp,
            tc.tile_pool(name="ep", bufs=2) as ep,
            tc.tile_pool(name="atp", bufs=2) as atp,
            tc.tile_pool(name="rzp", bufs=2) as rzp,
            tc.tile_pool(name="owp", bufs=2) as owp,
            tc.tile_pool(name="flatp", bufs=1) as flatp,
            tc.tile_pool(name="skp", bufs=4) as skp,
            tc.tile_pool(name="jsbp", bufs=3) as jsbp,
            tc.tile_pool(name="psj", bufs=3, space="PSUM") as psjp,
            tc.tile_pool(name="pss", bufs=1, space="PSUM") as pssp,
            tc.tile_pool(name="psmall", bufs=1, space="PSUM") as psmp,
        ):
            onescol = constp.tile([128, 1], BF16)
            nc.vector.memset(onescol[:, :], 1.0)
            onesrowS = constp.tile([1, 128], F32)
            nc.vector.memset(onesrowS[:, :], SCALE)

            def phase(b):
                # DMA order: QT + KT pieces first (they gate the scores
                # matmuls and everything downstream), then KV/KX bulk.
                QT = qtp.tile([128, NQ], BF16, tag="qt")
                nc.sync.dma_start(QT[:, :], qt[b])
                KT = ktp.tile([128, SEQ], BF16, tag="kt")
                KTPC = 4  # KT DMA pieces
                for piece in range(KTPC):
                    w = SEQ // KTPC
                    nc.sync.dma_start(KT[:, piece * w:(piece + 1) * w],
                                      kt[b, :, piece * w:(piece + 1) * w])

                # --- softmax numerator: E = exp(scale * K Q^T), 4 chunks/exp
                E = ep.tile([128, C * NQ], BF16, tag="e")
                for cc in range(C // 4):
                    ps_s = pssp.tile([128, 4 * NQ], F32, tag="scores")
                    for ci in range(4):
                        c = 4 * cc + ci
                        nc.tensor.matmul(ps_s[:, ci * NQ:(ci + 1) * NQ],
                                         KT[:, c * 128:(c + 1) * 128],
                                         QT[:, :], start=True, stop=True)
                    nc.scalar.activation(E[:, 4 * cc * NQ:(4 * cc + 4) * NQ],
                                         ps_s[:, :], AF.Exp,
                                         bias=0.0, scale=SCALE)

                KV = kvp.tile([128, C * 256], BF16, tag="kv")
                nc.sync.dma_start(KV[:, :].rearrange("p (c j) -> p c j", j=256),
                                  kvb[b].rearrange("c n j -> n c j"))
                KX = kvp.tile([128, C * 128], BF16, tag="kx")
                nc.sync.dma_start(KX[:, :].rearrange("p (c j) -> p c j", j=128),
                                  kvb[b, :, :, 128:256].rearrange("c n j -> n c j"))

                # --- Z (1 x NQ) = ones^T E ; rzb = SCALE/Z bcast to 128 parts
                ps_z = psmp.tile([128, 128], F32, tag="small")
                for c in range(C):
                    nc.tensor.matmul(ps_z[0:1, 0:NQ], onescol[:, :],
                                     E[:, c * NQ:(c + 1) * NQ],
                                     start=(c == 0), stop=(c == C - 1))
                rz = rzp.tile([1, NQ], F32, tag="rz")
                nc.vector.reciprocal(rz[:, :], ps_z[0:1, 0:NQ])
                ps_rzb = psmp.tile([128, 128], F32, tag="small")
                nc.tensor.matmul(ps_rzb[:, 0:NQ], onesrowS[:, :], rz[:, :],
                                 start=True, stop=True)

                # --- AT = E * rzb  (chunk-major layout [c*NQ + q])
                rzbsb = rzp.tile([128, NQ], F32, tag="rzbsb")
                nc.scalar.copy(rzbsb[:, :], ps_rzb[:, 0:NQ])
                AT = atp.tile([128, C * NQ], BF16, tag="at")
                H = C // 2
                for hh in range(2):
                    nc.gpsimd.tensor_mul(
                        AT[:, hh * H * NQ:(hh + 1) * H * NQ]
                        .rearrange("p (c q) -> p c q", q=NQ),
                        E[:, hh * H * NQ:(hh + 1) * H * NQ]
                        .rearrange("p (c q) -> p c q", q=NQ),
                        rzbsb[:, :].unsqueeze(1).broadcast_to((128, H, NQ)),
                    )

                # --- w rows [q, k] and o rows [q, v]  (both q-partition)
                ps_ow = psmp.tile([128, 128], F32, tag="small")
                for c in range(C):
                    nc.tensor.matmul(ps_ow[0:NQ, 0:128],
                                     AT[:, c * NQ:(c + 1) * NQ],
                                     KV[:, c * 256 + 128:(c + 1) * 256],
                                     start=(c == 0), stop=(c == C - 1))
                wsb = owp.tile([NQ, 128], BF16, tag="wsb")
                nc.scalar.copy(wsb[:, :], ps_ow[0:NQ, 0:128])
                wflat = flatp.tile([1, NQ * 128], BF16, tag="wflat")
                nc.sync.dma_start(
                    wflat[:, :].rearrange("o (q k) -> o q k", k=128),
                    wsb[:, :])

                ps_o = psmp.tile([128, 128], F32, tag="small")
                for c in range(C):
                    nc.tensor.matmul(ps_o[0:NQ, 0:128],
                                     AT[:, c * NQ:(c + 1) * NQ],
                                     KV[:, c * 256:c * 256 + 128],
                                     start=(c == 0), stop=(c == C - 1))
                negO = owp.tile([NQ, 128], BF16, tag="nego")
                nc.scalar.activation(negO[:, :], ps_o[0:NQ, 0:128],
                                     AF.Copy, bias=0.0, scale=-1.0 / SCALE)
                negOflat = flatp.tile([1, NQ * 128], BF16, tag="negoflat")
                nc.sync.dma_start(
                    negOflat[:, :].rearrange("o (q v) -> o q v", v=128),
                    negO[:, :])
                return dict(KV=KV, KX=KX, AT=AT, wflat=wflat,
                            negOflat=negOflat)

            QB = 2 * QG  # 16 q per block

            adup_tiles = {}

            def build_adup(ctx, b, blk):
                # pair-duplicated AT for a block's 16 q: [p, c, j, 2]
                AT = ctx["AT"]
                ADup = atp.tile([128, C * QB * 2], BF16, tag="adup")
                H = C // 2
                for hh in range(2):
                    nc.scalar.copy(
                        ADup[:, hh * H * QB * 2:(hh + 1) * H * QB * 2]
                        .rearrange("p (c j two) -> p c j two", j=QB, two=2),
                        AT[:, hh * H * NQ:(hh + 1) * H * NQ]
                        .rearrange("p (c q) -> p c q", q=NQ)
                        [:, :, blk * QB:(blk + 1) * QB].unsqueeze(3)
                        .broadcast_to((128, H, QB, 2)),
                    )
                adup_tiles[(b, blk)] = ADup

            def blocks(b, ctx, blk, nxt):
                KV, KX = ctx["KV"], ctx["KX"]
                wflat, negOflat = ctx["wflat"], ctx["negOflat"]
                ADup = adup_tiles.pop((b, blk))
                ps_a = psjp.tile([128, QG * 128], F32, tag="j")
                ps_b = psjp.tile([128, QG * 128], F32, tag="j")
                CU = 4  # chunks per SK unit
                for cc in range(C // CU):
                    c0 = CU * cc
                    sk = skp.tile([128, CU * QB * 128], BF16, tag="sk")
                    # dup-packed 2x_1P: both operands step-1 bf16
                    nc.vector.tensor_mul(
                        sk[:, :].rearrange(
                            "p (ci j kp two) -> p ci j kp two",
                            j=QB, kp=64, two=2),
                        KX[:, :].rearrange(
                            "p (c kp two) -> p c kp two", kp=64, two=2)
                        [:, c0:c0 + CU].unsqueeze(2)
                        .broadcast_to((128, CU, QB, 64, 2)),
                        ADup[:, :].rearrange(
                            "p (c j two) -> p c j two", j=QB, two=2)
                        [:, c0:c0 + CU].unsqueeze(3)
                        .broadcast_to((128, CU, QB, 64, 2)),
                    )
                    for ci in range(CU):
                        c = c0 + ci
                        for h, ps in enumerate((ps_a, ps_a, ps_b, ps_b)):
                            nc.tensor.matmul(
                                ps[:, (h % 2) * 512:(h % 2) * 512 + 512],
                                KV[:, c * 256:c * 256 + 128],
                                sk[:, ci * 2048 + h * 512:
                                   ci * 2048 + (h + 1) * 512],
                                start=(c == 0), stop=False,
                                skip_group_check=True)
                    if cc == 2 and nxt is not None:
                        # prebuild next block's ADup now, ahead of the jsb
                        # copies on the in-order ScalarE queue
                        build_adup(*nxt)
                for half, ps in enumerate((ps_a, ps_b)):
                    for j in range(QG):
                        q = blk * QB + half * QG + j
                        nc.tensor.matmul(ps[:, j * 128:(j + 1) * 128],
                                         negOflat[:, q * 128:(q + 1) * 128],
                                         wflat[:, q * 128:(q + 1) * 128],
                                         start=False, stop=True,
                                         skip_group_check=True,
                                         tile_position=(0, 0))
                    jsb = jsbp.tile([128, QG * 128], BF16, tag="jsb")
                    nc.scalar.copy(jsb[:, :], ps[:, :])
                    q0 = blk * QB + half * QG
                    nc.sync.dma_start(
                        out[b, q0:q0 + QG].rearrange("j v k -> v j k"),
                        jsb[:, :].rearrange("v (j k) -> v j k", k=128),
                    )

            # interleave: emit batch 1's phase right after batch 0's first
            # block so its AT is ready when Vector drains batch 0's SK.
            ctx0 = phase(0)
            ctx1 = phase(1)
            seq = [(ctx0, 0, blk) for blk in range(NQ // QB)]
            seq += [(ctx1, 1, blk) for blk in range(NQ // QB)]
            build_adup(*seq[0])
            for i, (ctx, b, blk) in enumerate(seq):
                nxt = seq[i + 1] if i + 1 < len(seq) else None
                blocks(b, ctx, blk, nxt)

    nc.compile()
    return nc


def _get_nc():
    if "nc" not in _CACHED:
        _CACHED["nc"] = _build()
    return _CACHED["nc"]


def _prep_core_inputs(query, keys, values, i):
    s = slice(i * BPC, (i + 1) * BPC)
    K = np.ascontiguousarray(keys[s])     # (2, 4096, 128) f32
    V = np.ascontiguousarray(values[s])
    Q = np.ascontiguousarray(query[s])    # (2, 64, 128) f32
    kvb = np.empty((BPC, C, 128, 256), dtype=ml_dtypes.bfloat16)
    kvb[:, :, :, 0:128] = V.reshape(BPC, C, 128, 128)
    kvb[:, :, :, 128:256] = K.reshape(BPC, C, 128, 128)
    kt = np.ascontiguousarray(K.transpose(0, 2, 1)).astype(ml_dtypes.bfloat16)
    qt = np.ascontiguousarray(Q.transpose(0, 2, 1)).astype(ml_dtypes.bfloat16)
    return {"kvb": kvb, "kt": kt, "qt": qt}


def kernel(query, keys, values):
    query = np.asarray(query, dtype=np.float32)
    keys = np.asarray(keys, dtype=np.float32)
    values = np.asarray(values, dtype=np.float32)
    nc = _get_nc()
    in_maps = [_prep_core_inputs(query, keys, values, i) for i in range(N_CORES)]
    res = run_bass_kernel_spmd(nc, in_maps, core_ids=list(range(N_CORES)))
    return np.concatenate(
        [np.asarray(res.results[i]["out"]) for i in range(N_CORES)],
        axis=0).astype(np.float32)


# revision 74
# speedup vs baseline: 1.0199x; 1.0014x over previous
"""AttentionJacobian kernel for 8 TRN2 NeuronCores.

J[b,q] = scale * ( V^T diag(a_q) K  -  o_q w_q^T ),  a = softmax(Q K^T scale)

Data-parallel over batch: 16 batches -> 2 per core. Per batch on-device:
  scoresT chunks (n x q) = KT_c^T @ QT        (bf16 matmuls)
  E = exp(scale * scoresT)                    (ScalarE, bf16; no max-sub
                                               needed: |scale*s| small)
  Z = ones^T @ E, rzb = SCALE/Z bcast         (PE + DVE reciprocal)
  AT = E * rzb                                (Pool, bf16, SCALE folded in)
  per 16-q block: ADup = AT pairs [p,c,j,2]   (ScalarE copy)
  SK units (4 chunks x 16 q): sk = K (*) a    as ONE DVE tensor_tensor in
    2x_1P packed mode -- both operands are step-1 bf16 reads (K pairs from
    a dedicated KX copy, a-pairs from ADup), 2 elem/lane/cycle.
    All SK on VectorE: Pool/ACT are 4x/3.5x slower per element and any
    GpSimd load trips the chip power duty-cycler (10.24us k-of-n windows),
    inflating every engine ~2.5x.
  term1 psum += V_c^T @ sk                    (bf16 matmuls, f32 psum,
                                               512-col groups, 8 banks)
  term2: rank-1 PE matmuls psum += (-o_q/SCALE) (x) w_q  (K=1, operands
    are [1,128] rows of SBUF-flattened w/negO at partition 0)
  evacuate psum -> bf16 jsb (ScalarE) -> HBM; host upcasts to f32.
Both batches' softmax phases are emitted before the block loops so the
Vector queue never stalls mid-stream; input DMAs are ordered QT/KT-pieces
first (they gate scores), then KV/KX bulk.
"""

import sys

for p in ("/opt/trn_rl_repo",):
    if p not in sys.path:
        sys.path.append(p)

import numpy as np
import ml_dtypes

import concourse.bass as bass
import concourse.bacc as bacc
import concourse.tile as tile
from concourse import mybir
from concourse.bass_utils import run_bass_kernel_spmd

N_CORES = 8
BATCH = 16
NQ = 64
SEQ = 4096
D = 128
BPC = BATCH // N_CORES        # batches per core = 2
C = SEQ // 128                # 32 contraction chunks
QG = 8                        # q per output group
NG = NQ // QG                 # 8 groups
SCALE = float(D) ** -0.5

F32 = mybir.dt.float32
BF16 = mybir.dt.bfloat16
AF = mybir.ActivationFunctionType
ALU = mybir.AluOpType

_CACHED = {}


def _build():
    nc = bacc.Bacc("TRN2", target_bir_lowering=False, debug=False,
                   num_devices=N_CORES)

    kvb = nc.dram_tensor("kvb", [BPC, C, 128, 256], BF16, kind="ExternalInput").ap()
    kt = nc.dram_tensor("kt", [BPC, 128, SEQ], BF16, kind="ExternalInput").ap()
    qt = nc.dram_tensor("qt", [BPC, 128, NQ], BF16, kind="ExternalInput").ap()
    # [b, v, q, k]: lets each output DMA write one contiguous 2KB block
    # per partition (128 descriptors vs 1024 for the [b,q,v,k] scatter);
    # host transposes to [b, q, v, k] after gather.
    out = nc.dram_tensor("out", [BPC, D, NQ, D], BF16, kind="ExternalOutput").ap()

    with tile.TileContext(nc) as tc:
        with (
            tc.tile_pool(name="const", bufs=1) as constp,
            tc.tile_pool(name="kv", bufs=2) as kvp,
            tc.tile_pool(name="ktp", bufs=2) as ktp,
            tc.tile_pool(name="qtp", bufs=2) as qtp,
            tc.tile_pool(name="ep", bufs=2) as ep,
            tc.tile_pool(name="atp", bufs=2) as atp,
            tc.tile_pool(name="rzp", bufs=2) as rzp,
            tc.tile_pool(name="owp", bufs=2) as owp,
            tc.tile_pool(name="flatp", bufs=1) as flatp,
            tc.tile_pool(name="skp", bufs=4) as skp,
            tc.tile_pool(name="jsbp", bufs=3) as jsbp,
            tc.tile_pool(name="psj", bufs=3, space="PSUM") as psjp,
            tc.tile_pool(name="pss", bufs=1, space="PSUM") as pssp,
            tc.tile_pool(name="psmall", bufs=1, space="PSUM") as psmp,
        ):
            onescol = constp.tile([128, 1], BF16)
            nc.vector.memset(onescol[:, :], 1.0)
            onesrowS = constp.tile([1, 128], F32)
            nc.vector.memset(onesrowS[:, :], SCALE)

            def phase(b):
                # DMA order: QT + KT pieces first (they gate the scores
                # matmuls and everything downstream), then KV/KX bulk.
                QT = qtp.tile([128, NQ], BF16, tag="qt")
                nc.sync.dma_start(QT[:, :], qt[b])
                KT = ktp.tile([128, SEQ], BF16, tag="kt")
                KTPC = 4  # KT DMA pieces
                for piece in range(KTPC):
                    w = SEQ // KTPC
                    nc.sync.dma_start(KT[:, piece * w:(piece + 1) * w],
                                      kt[b, :, piece * w:(piece + 1) * w])

                # --- softmax numerator: E = exp(scale * K Q^T), 4 chunks/exp
                E = ep.tile([128, C * NQ], BF16, tag="e")
                for cc in range(C // 4):
                    ps_s = pssp.tile([128, 4 * NQ], F32, tag="scores")
                    for ci in range(4):
                        c = 4 * cc + ci
                        nc.tensor.matmul(ps_s[:, ci * NQ:(ci + 1) * NQ],
                                         KT[:, c * 128:(c + 1) * 128],
                                         QT[:, :], start=True, stop=True)
                    nc.scalar.activation(E[:, 4 * cc * NQ:(4 * cc + 4) * NQ],
                                         ps_s[:, :], AF.Exp,
                                         bias=0.0, scale=SCALE)

                KV = kvp.tile([128, C * 256], BF16, tag="kv")
                nc.sync.dma_start(KV[:, :].rearrange("p (c j) -> p c j", j=256),
                                  kvb[b].rearrange("c n j -> n c j"))
                KX = kvp.tile([128, C * 128], BF16, tag="kx")
                nc.sync.dma_start(KX[:, :].rearrange("p (c j) -> p c j", j=128),
                                  kvb[b, :, :, 128:256].rearrange("c n j -> n c j"))

                # --- Z (1 x NQ) = ones^T E ; rzb = SCALE/Z bcast to 128 parts
                ps_z = psmp.tile([128, 128], F32, tag="small")
                for c in range(C):
                    nc.tensor.matmul(ps_z[0:1, 0:NQ], onescol[:, :],
                                     E[:, c * NQ:(c + 1) * NQ],
                                     start=(c == 0), stop=(c == C - 1))
                rz = rzp.tile([1, NQ], F32, tag="rz")
                nc.vector.reciprocal(rz[:, :], ps_z[0:1, 0:NQ])
                ps_rzb = psmp.tile([128, 128], F32, tag="small")
                nc.tensor.matmul(ps_rzb[:, 0:NQ], onesrowS[:, :], rz[:, :],
                                 start=True, stop=True)

                # --- AT = E * rzb  (chunk-major layout [c*NQ + q])
                rzbsb = rzp.tile([128, NQ], F32, tag="rzbsb")
                nc.scalar.copy(rzbsb[:, :], ps_rzb[:, 0:NQ])
                AT = atp.tile([128, C * NQ], BF16, tag="at")
                H = C // 2
                for hh in range(2):
                    nc.gpsimd.tensor_mul(
                        AT[:, hh * H * NQ:(hh + 1) * H * NQ]
                        .rearrange("p (c q) -> p c q", q=NQ),
                        E[:, hh * H * NQ:(hh + 1) * H * NQ]
                        .rearrange("p (c q) -> p c q", q=NQ),
                        rzbsb[:, :].unsqueeze(1).broadcast_to((128, H, NQ)),
                    )

                # --- w rows [q, k] and o rows [q, v]  (both q-partition)
                ps_ow = psmp.tile([128, 128], F32, tag="small")
                for c in range(C):
                    nc.tensor.matmul(ps_ow[0:NQ, 0:128],
                                     AT[:, c * NQ:(c + 1) * NQ],
                                     KV[:, c * 256 + 128:(c + 1) * 256],
                                     start=(c == 0), stop=(c == C - 1))
                wsb = owp.tile([NQ, 128], BF16, tag="wsb")
                nc.scalar.copy(wsb[:, :], ps_ow[0:NQ, 0:128])
                wflat = flatp.tile([1, NQ * 128], BF16, tag="wflat")
                nc.sync.dma_start(
                    wflat[:, :].rearrange("o (q k) -> o q k", k=128),
                    wsb[:, :])

                ps_o = psmp.tile([128, 128], F32, tag="small")
                for c in range(C):
                    nc.tensor.matmul(ps_o[0:NQ, 0:128],
                                     AT[:, c * NQ:(c + 1) * NQ],
                                     KV[:, c * 256:c * 256 + 128],
                                     start=(c == 0), stop=(c == C - 1))
                negO = owp.tile([NQ, 128], BF16, tag="nego")
                nc.scalar.activation(negO[:, :], ps_o[0:NQ, 0:128],
                                     AF.Copy, bias=0.0, scale=-1.0 / SCALE)
                negOflat = flatp.tile([1, NQ * 128], BF16, tag="negoflat")
                nc.sync.dma_start(
                    negOflat[:, :].rearrange("o (q v) -> o q v", v=128),
                    negO[:, :])
                return dict(KV=KV, KX=KX, AT=AT, wflat=wflat,
                            negOflat=negOflat)

            QB = 2 * QG  # 16 q per block

            adup_tiles = {}

            def build_adup(ctx, b, blk):
                # pair-duplicated AT for a block's 16 q: [p, c, j, 2]
                AT = ctx["AT"]
                ADup = atp.tile([128, C * QB * 2], BF16, tag="adup")
                H = C // 2
                for hh in range(2):
                    nc.scalar.copy(
                        ADup[:, hh * H * QB * 2:(hh + 1) * H * QB * 2]
                        .rearrange("p (c j two) -> p c j two", j=QB, two=2),
                        AT[:, hh * H * NQ:(hh + 1) * H * NQ]
                        .rearrange("p (c q) -> p c q", q=NQ)
                        [:, :, blk * QB:(blk + 1) * QB].unsqueeze(3)
                        .broadcast_to((128, H, QB, 2)),
                    )
                adup_tiles[(b, blk)] = ADup

            def blocks(b, ctx, blk, nxt):
                KV, KX = ctx["KV"], ctx["KX"]
                wflat, negOflat = ctx["wflat"], ctx["negOflat"]
                ADup = adup_tiles.pop((b, blk))
                ps_a = psjp.tile([128, QG * 128], F32, tag="j")
                ps_b = psjp.tile([128, QG * 128], F32, tag="j")
                CU = 4  # chunks per SK unit
                for cc in range(C // CU):
                    c0 = CU * cc
                    sk = skp.tile([128, CU * QB * 128], BF16, tag="sk")
                    # dup-packed 2x_1P: both operands step-1 bf16
                    nc.vector.tensor_mul(
                        sk[:, :].rearrange(
                            "p (ci j kp two) -> p ci j kp two",
                            j=QB, kp=64, two=2),
                        KX[:, :].rearrange(
                            "p (c kp two) -> p c kp two", kp=64, two=2)
                        [:, c0:c0 + CU].unsqueeze(2)
                        .broadcast_to((128, CU, QB, 64, 2)),
                        ADup[:, :].rearrange(
                            "p (c j two) -> p c j two", j=QB, two=2)
                        [:, c0:c0 + CU].unsqueeze(3)
                        .broadcast_to((128, CU, QB, 64, 2)),
                    )
                    for ci in range(CU):
                        c = c0 + ci
                        for h, ps in enumerate((ps_a, ps_a, ps_b, ps_b)):
                            nc.tensor.matmul(
                                ps[:, (h % 2) * 512:(h % 2) * 512 + 512],
                                KV[:, c * 256:c * 256 + 128],
                                sk[:, ci * 2048 + h * 512:
                                   ci * 2048 + (h + 1) * 512],
                                start=(c == 0), stop=False,
                                skip_group_check=True)
                    if cc == 2 and nxt is not None:
                        # prebuild next block's ADup now, ahead of the jsb
                        # copies on the in-order ScalarE queue
                        build_adup(*nxt)
                for half, ps in enumerate((ps_a, ps_b)):
                    for j in range(QG):
                        q = blk * QB + half * QG + j
                        nc.tensor.matmul(ps[:, j * 128:(j + 1) * 128],
                                         negOflat[:, q * 128:(q + 1) * 128],
                                         wflat[:, q * 128:(q + 1) * 128],
                                         start=False, stop=True,
                                         skip_group_check=True,
                                         tile_position=(0, 0))
                    jsb = jsbp.tile([128, QG * 128], BF16, tag="jsb")
                    nc.scalar.copy(jsb[:, :], ps[:, :])
                    q0 = blk * QB + half * QG
                    nc.sync.dma_start(
                        out[b, :, q0:q0 + QG, :].rearrange("v j k -> v (j k)"),
                        jsb[:, :],
                    )

            # interleave: emit batch 1's phase right after batch 0's first
            # block so its AT is ready when Vector drains batch 0's SK.
            ctx0 = phase(0)
            ctx1 = phase(1)
            seq = [(ctx0, 0, blk) for blk in range(NQ // QB)]
            seq += [(ctx1, 1, blk) for blk in range(NQ // QB)]
            build_adup(*seq[0])
            for i, (ctx, b, blk) in enumerate(seq):
                nxt = seq[i + 1] if i + 1 < len(seq) else None
                blocks(b, ctx, blk, nxt)

    nc.compile()
    return nc


def _get_nc():
    if "nc" not in _CACHED:
        _CACHED["nc"] = _build()
    return _CACHED["nc"]


def _prep_core_inputs(query, keys, values, i):
    s = slice(i * BPC, (i + 1) * BPC)
    K = np.ascontiguousarray(keys[s])     # (2, 4096, 128) f32
    V = np.ascontiguousarray(values[s])
    Q = np.ascontiguousarray(query[s])    # (2, 64, 128) f32
    kvb = np.empty((BPC, C, 128, 256), dtype=ml_dtypes.bfloat16)
    kvb[:, :, :, 0:128] = V.reshape(BPC, C, 128, 128)
    kvb[:, :, :, 128:256] = K.reshape(BPC, C, 128, 128)
    kt = np.ascontiguousarray(K.transpose(0, 2, 1)).astype(ml_dtypes.bfloat16)
    qt = np.ascontiguousarray(Q.transpose(0, 2, 1)).astype(ml_dtypes.bfloat16)
    return {"kvb": kvb, "kt": kt, "qt": qt}


def kernel(query, keys, values):
    query = np.asarray(query, dtype=np.float32)
    keys = np.asarray(keys, dtype=np.float32)
    values = np.asarray(values, dtype=np.float32)
    nc = _get_nc()
    in_maps = [_prep_core_inputs(query, keys, values, i) for i in range(N_CORES)]
    res = run_bass_kernel_spmd(nc, in_maps, core_ids=list(range(N_CORES)))
    full = np.concatenate(
        [np.asarray(res.results[i]["out"]) for i in range(N_CORES)],
        axis=0)                      # (16, 128v, 64q, 128k)
    return np.ascontiguousarray(full.transpose(0, 2, 1, 3)).astype(np.float32)


# revision 75
# speedup vs baseline: 1.0250x; 1.0049x over previous
"""AttentionJacobian kernel for 8 TRN2 NeuronCores.

J[b,q] = scale * ( V^T diag(a_q) K  -  o_q w_q^T ),  a = softmax(Q K^T scale)

Data-parallel over batch: 16 batches -> 2 per core. Per batch on-device:
  scoresT chunks (n x q) = KT_c^T @ QT        (bf16 matmuls)
  E = exp(scale * scoresT)                    (ScalarE, bf16; no max-sub
                                               needed: |scale*s| small)
  Z = ones^T @ E, rzb = SCALE/Z bcast         (PE + DVE reciprocal)
  AT = E * rzb                                (Pool, bf16, SCALE folded in)
  per 16-q block: ADup = AT pairs [p,c,j,2]   (ScalarE copy)
  SK units (4 chunks x 16 q): sk = K (*) a    as ONE DVE tensor_tensor in
    2x_1P packed mode -- both operands are step-1 bf16 reads (K pairs from
    a dedicated KX copy, a-pairs from ADup), 2 elem/lane/cycle.
    All SK on VectorE: Pool/ACT are 4x/3.5x slower per element and any
    GpSimd load trips the chip power duty-cycler (10.24us k-of-n windows),
    inflating every engine ~2.5x.
  term1 psum += V_c^T @ sk                    (bf16 matmuls, f32 psum,
                                               512-col groups, 8 banks)
  term2: rank-1 PE matmuls psum += (-o_q/SCALE) (x) w_q  (K=1, operands
    are [1,128] rows of SBUF-flattened w/negO at partition 0)
  evacuate psum -> bf16 jsb (ScalarE) -> HBM; host upcasts to f32.
Both batches' softmax phases are emitted before the block loops so the
Vector queue never stalls mid-stream; input DMAs are ordered QT/KT-pieces
first (they gate scores), then KV/KX bulk.
"""

import sys

for p in ("/opt/trn_rl_repo",):
    if p not in sys.path:
        sys.path.append(p)

import numpy as np
import ml_dtypes

import concourse.bass as bass
import concourse.bacc as bacc
import concourse.tile as tile
from concourse import mybir
from concourse.bass_utils import run_bass_kernel_spmd

N_CORES = 8
BATCH = 16
NQ = 64
SEQ = 4096
D = 128
BPC = BATCH // N_CORES        # batches per core = 2
C = SEQ // 128                # 32 contraction chunks
QG = 8                        # q per output group
NG = NQ // QG                 # 8 groups
SCALE = float(D) ** -0.5

F32 = mybir.dt.float32
BF16 = mybir.dt.bfloat16
AF = mybir.ActivationFunctionType
ALU = mybir.AluOpType

_CACHED = {}


def _build():
    nc = bacc.Bacc("TRN2", target_bir_lowering=False, debug=False,
                   num_devices=N_CORES)

    kvb = nc.dram_tensor("kvb", [BPC, C, 128, 256], BF16, kind="ExternalInput").ap()
    kt = nc.dram_tensor("kt", [BPC, 128, SEQ], BF16, kind="ExternalInput").ap()
    qt = nc.dram_tensor("qt", [BPC, 128, NQ], BF16, kind="ExternalInput").ap()
    # [b, v, q, k]: lets each output DMA write one contiguous 2KB block
    # per partition (128 descriptors vs 1024 for the [b,q,v,k] scatter);
    # host transposes to [b, q, v, k] after gather.
    out = nc.dram_tensor("out", [BPC, D, NQ, D], BF16, kind="ExternalOutput").ap()

    with tile.TileContext(nc) as tc:
        with (
            tc.tile_pool(name="const", bufs=1) as constp,
            tc.tile_pool(name="kv", bufs=2) as kvp,
            tc.tile_pool(name="ktp", bufs=2) as ktp,
            tc.tile_pool(name="qtp", bufs=2) as qtp,
            tc.tile_pool(name="ep", bufs=2) as ep,
            tc.tile_pool(name="atp", bufs=2) as atp,
            tc.tile_pool(name="rzp", bufs=2) as rzp,
            tc.tile_pool(name="owp", bufs=2) as owp,
            tc.tile_pool(name="flatp", bufs=1) as flatp,
            tc.tile_pool(name="skp", bufs=4) as skp,
            tc.tile_pool(name="jsbp", bufs=3) as jsbp,
            tc.tile_pool(name="psj", bufs=3, space="PSUM") as psjp,
            tc.tile_pool(name="pss", bufs=1, space="PSUM") as pssp,
            tc.tile_pool(name="psmall", bufs=1, space="PSUM") as psmp,
        ):
            onescol = constp.tile([128, 1], BF16)
            nc.vector.memset(onescol[:, :], 1.0)
            onesrowS = constp.tile([1, 128], F32)
            nc.vector.memset(onesrowS[:, :], SCALE)

            def phase(b):
                # DMA order: QT + KT pieces first (they gate the scores
                # matmuls and everything downstream), then KV/KX bulk.
                QT = qtp.tile([128, NQ], BF16, tag="qt")
                nc.sync.dma_start(QT[:, :], qt[b])
                KT = ktp.tile([128, SEQ], BF16, tag="kt")
                KTPC = 4  # KT DMA pieces
                for piece in range(KTPC):
                    w = SEQ // KTPC
                    nc.sync.dma_start(KT[:, piece * w:(piece + 1) * w],
                                      kt[b, :, piece * w:(piece + 1) * w])

                # --- softmax numerator: E = exp(scale * K Q^T), 4 chunks/exp
                E = ep.tile([128, C * NQ], BF16, tag="e")
                for cc in range(C // 4):
                    ps_s = pssp.tile([128, 4 * NQ], F32, tag="scores")
                    for ci in range(4):
                        c = 4 * cc + ci
                        nc.tensor.matmul(ps_s[:, ci * NQ:(ci + 1) * NQ],
                                         KT[:, c * 128:(c + 1) * 128],
                                         QT[:, :], start=True, stop=True)
                    nc.scalar.activation(E[:, 4 * cc * NQ:(4 * cc + 4) * NQ],
                                         ps_s[:, :], AF.Exp,
                                         bias=0.0, scale=SCALE)

                KV = kvp.tile([128, C * 256], BF16, tag="kv")
                nc.sync.dma_start(KV[:, :].rearrange("p (c j) -> p c j", j=256),
                                  kvb[b].rearrange("c n j -> n c j"))

                # --- Z (1 x NQ) = ones^T E ; rzb = SCALE/Z bcast to 128 parts
                ps_z = psmp.tile([128, 128], F32, tag="small")
                for c in range(C):
                    nc.tensor.matmul(ps_z[0:1, 0:NQ], onescol[:, :],
                                     E[:, c * NQ:(c + 1) * NQ],
                                     start=(c == 0), stop=(c == C - 1))
                rz = rzp.tile([1, NQ], F32, tag="rz")
                nc.vector.reciprocal(rz[:, :], ps_z[0:1, 0:NQ])
                ps_rzb = psmp.tile([128, 128], F32, tag="small")
                nc.tensor.matmul(ps_rzb[:, 0:NQ], onesrowS[:, :], rz[:, :],
                                 start=True, stop=True)

                # --- AT = E * rzb  (chunk-major layout [c*NQ + q])
                rzbsb = rzp.tile([128, NQ], F32, tag="rzbsb")
                nc.scalar.copy(rzbsb[:, :], ps_rzb[:, 0:NQ])
                AT = atp.tile([128, C * NQ], BF16, tag="at")
                H = C // 2
                for hh in range(2):
                    nc.gpsimd.tensor_mul(
                        AT[:, hh * H * NQ:(hh + 1) * H * NQ]
                        .rearrange("p (c q) -> p c q", q=NQ),
                        E[:, hh * H * NQ:(hh + 1) * H * NQ]
                        .rearrange("p (c q) -> p c q", q=NQ),
                        rzbsb[:, :].unsqueeze(1).broadcast_to((128, H, NQ)),
                    )

                # --- w rows [q, k] and o rows [q, v]  (both q-partition)
                ps_ow = psmp.tile([128, 128], F32, tag="small")
                for c in range(C):
                    nc.tensor.matmul(ps_ow[0:NQ, 0:128],
                                     AT[:, c * NQ:(c + 1) * NQ],
                                     KV[:, c * 256 + 128:(c + 1) * 256],
                                     start=(c == 0), stop=(c == C - 1))
                wsb = owp.tile([NQ, 128], BF16, tag="wsb")
                nc.scalar.copy(wsb[:, :], ps_ow[0:NQ, 0:128])
                wflat = flatp.tile([1, NQ * 128], BF16, tag="wflat")
                nc.sync.dma_start(
                    wflat[:, :].rearrange("o (q k) -> o q k", k=128),
                    wsb[:, :])

                ps_o = psmp.tile([128, 128], F32, tag="small")
                for c in range(C):
                    nc.tensor.matmul(ps_o[0:NQ, 0:128],
                                     AT[:, c * NQ:(c + 1) * NQ],
                                     KV[:, c * 256:c * 256 + 128],
                                     start=(c == 0), stop=(c == C - 1))
                negO = owp.tile([NQ, 128], BF16, tag="nego")
                nc.scalar.activation(negO[:, :], ps_o[0:NQ, 0:128],
                                     AF.Copy, bias=0.0, scale=-1.0 / SCALE)
                negOflat = flatp.tile([1, NQ * 128], BF16, tag="negoflat")
                nc.sync.dma_start(
                    negOflat[:, :].rearrange("o (q v) -> o q v", v=128),
                    negO[:, :])
                return dict(KV=KV, KX=KX, AT=AT, wflat=wflat,
                            negOflat=negOflat)

            QB = 2 * QG  # 16 q per block

            adup_tiles = {}

            def build_adup(ctx, b, blk):
                # pair-duplicated AT for a block's 16 q: [p, c, j, 2]
                AT = ctx["AT"]
                ADup = atp.tile([128, C * QB * 2], BF16, tag="adup")
                H = C // 2
                for hh in range(2):
                    nc.scalar.copy(
                        ADup[:, hh * H * QB * 2:(hh + 1) * H * QB * 2]
                        .rearrange("p (c j two) -> p c j two", j=QB, two=2),
                        AT[:, hh * H * NQ:(hh + 1) * H * NQ]
                        .rearrange("p (c q) -> p c q", q=NQ)
                        [:, :, blk * QB:(blk + 1) * QB].unsqueeze(3)
                        .broadcast_to((128, H, QB, 2)),
                    )
                adup_tiles[(b, blk)] = ADup

            def blocks(b, ctx, blk, nxt):
                KV, KX = ctx["KV"], ctx["KX"]
                wflat, negOflat = ctx["wflat"], ctx["negOflat"]
                ADup = adup_tiles.pop((b, blk))
                ps_a = psjp.tile([128, QG * 128], F32, tag="j")
                ps_b = psjp.tile([128, QG * 128], F32, tag="j")
                CU = 4  # chunks per SK unit
                for cc in range(C // CU):
                    c0 = CU * cc
                    sk = skp.tile([128, CU * QB * 128], BF16, tag="sk")
                    # dup-packed 2x_1P: both operands step-1 bf16
                    nc.vector.tensor_mul(
                        sk[:, :].rearrange(
                            "p (ci j kp two) -> p ci j kp two",
                            j=QB, kp=64, two=2),
                        KV[:, :].rearrange(
                            "p (c half kp two) -> p c half kp two",
                            half=2, kp=64, two=2)
                        [:, c0:c0 + CU, 1].unsqueeze(2)
                        .broadcast_to((128, CU, QB, 64, 2)),
                        ADup[:, :].rearrange(
                            "p (c j two) -> p c j two", j=QB, two=2)
                        [:, c0:c0 + CU].unsqueeze(3)
                        .broadcast_to((128, CU, QB, 64, 2)),
                    )
                    for ci in range(CU):
                        c = c0 + ci
                        for h, ps in enumerate((ps_a, ps_a, ps_b, ps_b)):
                            nc.tensor.matmul(
                                ps[:, (h % 2) * 512:(h % 2) * 512 + 512],
                                KV[:, c * 256:c * 256 + 128],
                                sk[:, ci * 2048 + h * 512:
                                   ci * 2048 + (h + 1) * 512],
                                start=(c == 0), stop=False,
                                skip_group_check=True)
                    if cc == 2 and nxt is not None:
                        # prebuild next block's ADup now, ahead of the jsb
                        # copies on the in-order ScalarE queue
                        build_adup(*nxt)
                for half, ps in enumerate((ps_a, ps_b)):
                    for j in range(QG):
                        q = blk * QB + half * QG + j
                        nc.tensor.matmul(ps[:, j * 128:(j + 1) * 128],
                                         negOflat[:, q * 128:(q + 1) * 128],
                                         wflat[:, q * 128:(q + 1) * 128],
                                         start=False, stop=True,
                                         skip_group_check=True,
                                         tile_position=(0, 0))
                    jsb = jsbp.tile([128, QG * 128], BF16, tag="jsb")
                    nc.scalar.copy(jsb[:, :], ps[:, :])
                    q0 = blk * QB + half * QG
                    nc.sync.dma_start(
                        out[b, :, q0:q0 + QG, :].rearrange("v j k -> v (j k)"),
                        jsb[:, :],
                    )

            # interleave: emit batch 1's phase right after batch 0's first
            # block so its AT is ready when Vector drains batch 0's SK.
            ctx0 = phase(0)
            ctx1 = phase(1)
            seq = [(ctx0, 0, blk) for blk in range(NQ // QB)]
            seq += [(ctx1, 1, blk) for blk in range(NQ // QB)]
            build_adup(*seq[0])
            for i, (ctx, b, blk) in enumerate(seq):
                nxt = seq[i + 1] if i + 1 < len(seq) else None
                blocks(b, ctx, blk, nxt)

    nc.compile()
    return nc


def _get_nc():
    if "nc" not in _CACHED:
        _CACHED["nc"] = _build()
    return _CACHED["nc"]


def _prep_core_inputs(query, keys, values, i):
    s = slice(i * BPC, (i + 1) * BPC)
    K = np.ascontiguousarray(keys[s])     # (2, 4096, 128) f32
    V = np.ascontiguousarray(values[s])
    Q = np.ascontiguousarray(query[s])    # (2, 64, 128) f32
    kvb = np.empty((BPC, C, 128, 256), dtype=ml_dtypes.bfloat16)
    kvb[:, :, :, 0:128] = V.reshape(BPC, C, 128, 128)
    kvb[:, :, :, 128:256] = K.reshape(BPC, C, 128, 128)
    kt = np.ascontiguousarray(K.transpose(0, 2, 1)).astype(ml_dtypes.bfloat16)
    qt = np.ascontiguousarray(Q.transpose(0, 2, 1)).astype(ml_dtypes.bfloat16)
    return {"kvb": kvb, "kt": kt, "qt": qt}


def kernel(query, keys, values):
    query = np.asarray(query, dtype=np.float32)
    keys = np.asarray(keys, dtype=np.float32)
    values = np.asarray(values, dtype=np.float32)
    nc = _get_nc()
    in_maps = [_prep_core_inputs(query, keys, values, i) for i in range(N_CORES)]
    res = run_bass_kernel_spmd(nc, in_maps, core_ids=list(range(N_CORES)))
    full = np.concatenate(
        [np.asarray(res.results[i]["out"]) for i in range(N_CORES)],
        axis=0)                      # (16, 128v, 64q, 128k)
    return np.ascontiguousarray(full.transpose(0, 2, 1, 3)).astype(np.float32)


# revision 76
# speedup vs baseline: 1.0253x; 1.0003x over previous
"""AttentionJacobian kernel for 8 TRN2 NeuronCores.

J[b,q] = scale * ( V^T diag(a_q) K  -  o_q w_q^T ),  a = softmax(Q K^T scale)

Data-parallel over batch: 16 batches -> 2 per core. Per batch on-device:
  scoresT chunks (n x q) = KT_c^T @ QT        (bf16 matmuls)
  E = exp(scale * scoresT)                    (ScalarE, bf16; no max-sub
                                               needed: |scale*s| small)
  Z = ones^T @ E, rzb = SCALE/Z bcast         (PE + DVE reciprocal)
  AT = E * rzb                                (Pool, bf16, SCALE folded in)
  per 16-q block: ADup = AT pairs [p,c,j,2]   (ScalarE copy)
  SK units (4 chunks x 16 q): sk = K (*) a    as ONE DVE tensor_tensor in
    2x_1P packed mode -- both operands are step-1 bf16 reads (K pairs from
    a dedicated KX copy, a-pairs from ADup), 2 elem/lane/cycle.
    All SK on VectorE: Pool/ACT are 4x/3.5x slower per element and any
    GpSimd load trips the chip power duty-cycler (10.24us k-of-n windows),
    inflating every engine ~2.5x.
  term1 psum += V_c^T @ sk                    (bf16 matmuls, f32 psum,
                                               512-col groups, 8 banks)
  term2: rank-1 PE matmuls psum += (-o_q/SCALE) (x) w_q  (K=1, operands
    are [1,128] rows of SBUF-flattened w/negO at partition 0)
  evacuate psum -> bf16 jsb (ScalarE) -> HBM; host upcasts to f32.
Both batches' softmax phases are emitted before the block loops so the
Vector queue never stalls mid-stream; input DMAs are ordered QT/KT-pieces
first (they gate scores), then KV/KX bulk.
"""

import sys

for p in ("/opt/trn_rl_repo",):
    if p not in sys.path:
        sys.path.append(p)

import numpy as np
import ml_dtypes

import concourse.bass as bass
import concourse.bacc as bacc
import concourse.tile as tile
from concourse import mybir
from concourse.bass_utils import run_bass_kernel_spmd

N_CORES = 8
BATCH = 16
NQ = 64
SEQ = 4096
D = 128
BPC = BATCH // N_CORES        # batches per core = 2
C = SEQ // 128                # 32 contraction chunks
QG = 8                        # q per output group
NG = NQ // QG                 # 8 groups
SCALE = float(D) ** -0.5

F32 = mybir.dt.float32
BF16 = mybir.dt.bfloat16
AF = mybir.ActivationFunctionType
ALU = mybir.AluOpType

_CACHED = {}


def _build():
    nc = bacc.Bacc("TRN2", target_bir_lowering=False, debug=False,
                   num_devices=N_CORES)

    kvb = nc.dram_tensor("kvb", [BPC, C, 128, 256], BF16, kind="ExternalInput").ap()
    kt = nc.dram_tensor("kt", [BPC, 128, SEQ], BF16, kind="ExternalInput").ap()
    qt = nc.dram_tensor("qt", [BPC, 128, NQ], BF16, kind="ExternalInput").ap()
    # [b, v, q, k]: lets each output DMA write one contiguous 2KB block
    # per partition (128 descriptors vs 1024 for the [b,q,v,k] scatter);
    # host transposes to [b, q, v, k] after gather.
    out = nc.dram_tensor("out", [BPC, D, NQ, D], BF16, kind="ExternalOutput").ap()

    with tile.TileContext(nc) as tc:
        with (
            tc.tile_pool(name="const", bufs=1) as constp,
            tc.tile_pool(name="kv", bufs=2) as kvp,
            tc.tile_pool(name="ktp", bufs=2) as ktp,
            tc.tile_pool(name="qtp", bufs=2) as qtp,
            tc.tile_pool(name="ep", bufs=2) as ep,
            tc.tile_pool(name="atp", bufs=2) as atp,
            tc.tile_pool(name="rzp", bufs=2) as rzp,
            tc.tile_pool(name="owp", bufs=2) as owp,
            tc.tile_pool(name="flatp", bufs=2) as flatp,
            tc.tile_pool(name="skp", bufs=4) as skp,
            tc.tile_pool(name="jsbp", bufs=3) as jsbp,
            tc.tile_pool(name="psj", bufs=3, space="PSUM") as psjp,
            tc.tile_pool(name="pss", bufs=1, space="PSUM") as pssp,
            tc.tile_pool(name="psmall", bufs=1, space="PSUM") as psmp,
        ):
            onescol = constp.tile([128, 1], BF16)
            nc.vector.memset(onescol[:, :], 1.0)
            onesrowS = constp.tile([1, 128], F32)
            nc.vector.memset(onesrowS[:, :], SCALE)

            def phase(b):
                # DMA order: QT + KT pieces first (they gate the scores
                # matmuls and everything downstream), then KV/KX bulk.
                QT = qtp.tile([128, NQ], BF16, tag="qt")
                nc.sync.dma_start(QT[:, :], qt[b])
                KT = ktp.tile([128, SEQ], BF16, tag="kt")
                KTPC = 4  # KT DMA pieces
                for piece in range(KTPC):
                    w = SEQ // KTPC
                    nc.sync.dma_start(KT[:, piece * w:(piece + 1) * w],
                                      kt[b, :, piece * w:(piece + 1) * w])

                # --- softmax numerator: E = exp(scale * K Q^T), 4 chunks/exp
                E = ep.tile([128, C * NQ], BF16, tag="e")
                for cc in range(C // 4):
                    ps_s = pssp.tile([128, 4 * NQ], F32, tag="scores")
                    for ci in range(4):
                        c = 4 * cc + ci
                        nc.tensor.matmul(ps_s[:, ci * NQ:(ci + 1) * NQ],
                                         KT[:, c * 128:(c + 1) * 128],
                                         QT[:, :], start=True, stop=True)
                    nc.scalar.activation(E[:, 4 * cc * NQ:(4 * cc + 4) * NQ],
                                         ps_s[:, :], AF.Exp,
                                         bias=0.0, scale=SCALE)

                KV = kvp.tile([128, C * 256], BF16, tag="kv")
                nc.sync.dma_start(KV[:, :].rearrange("p (c j) -> p c j", j=256),
                                  kvb[b].rearrange("c n j -> n c j"))

                # --- Z (1 x NQ) = ones^T E ; rzb = SCALE/Z bcast to 128 parts
                ps_z = psmp.tile([128, 128], F32, tag="small")
                for c in range(C):
                    nc.tensor.matmul(ps_z[0:1, 0:NQ], onescol[:, :],
                                     E[:, c * NQ:(c + 1) * NQ],
                                     start=(c == 0), stop=(c == C - 1))
                rz = rzp.tile([1, NQ], F32, tag="rz")
                nc.vector.reciprocal(rz[:, :], ps_z[0:1, 0:NQ])
                ps_rzb = psmp.tile([128, 128], F32, tag="small")
                nc.tensor.matmul(ps_rzb[:, 0:NQ], onesrowS[:, :], rz[:, :],
                                 start=True, stop=True)

                # --- AT = E * rzb  (chunk-major layout [c*NQ + q])
                rzbsb = rzp.tile([128, NQ], F32, tag="rzbsb")
                nc.scalar.copy(rzbsb[:, :], ps_rzb[:, 0:NQ])
                AT = atp.tile([128, C * NQ], BF16, tag="at")
                H = C // 2
                for hh in range(2):
                    nc.gpsimd.tensor_mul(
                        AT[:, hh * H * NQ:(hh + 1) * H * NQ]
                        .rearrange("p (c q) -> p c q", q=NQ),
                        E[:, hh * H * NQ:(hh + 1) * H * NQ]
                        .rearrange("p (c q) -> p c q", q=NQ),
                        rzbsb[:, :].unsqueeze(1).broadcast_to((128, H, NQ)),
                    )

                # --- w rows [q, k] and o rows [q, v]  (both q-partition)
                ps_ow = psmp.tile([128, 128], F32, tag="small")
                for c in range(C):
                    nc.tensor.matmul(ps_ow[0:NQ, 0:128],
                                     AT[:, c * NQ:(c + 1) * NQ],
                                     KV[:, c * 256 + 128:(c + 1) * 256],
                                     start=(c == 0), stop=(c == C - 1))
                wsb = owp.tile([NQ, 128], BF16, tag="wsb")
                nc.scalar.copy(wsb[:, :], ps_ow[0:NQ, 0:128])
                wflat = flatp.tile([1, NQ * 128], BF16, tag="wflat")
                nc.sync.dma_start(
                    wflat[:, :].rearrange("o (q k) -> o q k", k=128),
                    wsb[:, :])

                ps_o = psmp.tile([128, 128], F32, tag="small")
                for c in range(C):
                    nc.tensor.matmul(ps_o[0:NQ, 0:128],
                                     AT[:, c * NQ:(c + 1) * NQ],
                                     KV[:, c * 256:c * 256 + 128],
                                     start=(c == 0), stop=(c == C - 1))
                negO = owp.tile([NQ, 128], BF16, tag="nego")
                nc.scalar.activation(negO[:, :], ps_o[0:NQ, 0:128],
                                     AF.Copy, bias=0.0, scale=-1.0 / SCALE)
                negOflat = flatp.tile([1, NQ * 128], BF16, tag="negoflat")
                nc.sync.dma_start(
                    negOflat[:, :].rearrange("o (q v) -> o q v", v=128),
                    negO[:, :])
                return dict(KV=KV, KX=KX, AT=AT, wflat=wflat,
                            negOflat=negOflat)

            QB = 2 * QG  # 16 q per block

            adup_tiles = {}

            def build_adup(ctx, b, blk):
                # pair-duplicated AT for a block's 16 q: [p, c, j, 2]
                AT = ctx["AT"]
                ADup = atp.tile([128, C * QB * 2], BF16, tag="adup")
                H = C // 2
                for hh in range(2):
                    nc.scalar.copy(
                        ADup[:, hh * H * QB * 2:(hh + 1) * H * QB * 2]
                        .rearrange("p (c j two) -> p c j two", j=QB, two=2),
                        AT[:, hh * H * NQ:(hh + 1) * H * NQ]
                        .rearrange("p (c q) -> p c q", q=NQ)
                        [:, :, blk * QB:(blk + 1) * QB].unsqueeze(3)
                        .broadcast_to((128, H, QB, 2)),
                    )
                adup_tiles[(b, blk)] = ADup

            def blocks(b, ctx, blk, nxt):
                KV, KX = ctx["KV"], ctx["KX"]
                wflat, negOflat = ctx["wflat"], ctx["negOflat"]
                ADup = adup_tiles.pop((b, blk))
                ps_a = psjp.tile([128, QG * 128], F32, tag="j")
                ps_b = psjp.tile([128, QG * 128], F32, tag="j")
                CU = 4  # chunks per SK unit
                for cc in range(C // CU):
                    c0 = CU * cc
                    sk = skp.tile([128, CU * QB * 128], BF16, tag="sk")
                    # dup-packed 2x_1P: both operands step-1 bf16
                    nc.vector.tensor_mul(
                        sk[:, :].rearrange(
                            "p (ci j kp two) -> p ci j kp two",
                            j=QB, kp=64, two=2),
                        KV[:, :].rearrange(
                            "p (c half kp two) -> p c half kp two",
                            half=2, kp=64, two=2)
                        [:, c0:c0 + CU, 1].unsqueeze(2)
                        .broadcast_to((128, CU, QB, 64, 2)),
                        ADup[:, :].rearrange(
                            "p (c j two) -> p c j two", j=QB, two=2)
                        [:, c0:c0 + CU].unsqueeze(3)
                        .broadcast_to((128, CU, QB, 64, 2)),
                    )
                    for ci in range(CU):
                        c = c0 + ci
                        for h, ps in enumerate((ps_a, ps_a, ps_b, ps_b)):
                            nc.tensor.matmul(
                                ps[:, (h % 2) * 512:(h % 2) * 512 + 512],
                                KV[:, c * 256:c * 256 + 128],
                                sk[:, ci * 2048 + h * 512:
                                   ci * 2048 + (h + 1) * 512],
                                start=(c == 0), stop=False,
                                skip_group_check=True)
                    if cc == 2 and nxt is not None:
                        # prebuild next block's ADup now, ahead of the jsb
                        # copies on the in-order ScalarE queue
                        build_adup(*nxt)
                for half, ps in enumerate((ps_a, ps_b)):
                    for j in range(QG):
                        q = blk * QB + half * QG + j
                        nc.tensor.matmul(ps[:, j * 128:(j + 1) * 128],
                                         negOflat[:, q * 128:(q + 1) * 128],
                                         wflat[:, q * 128:(q + 1) * 128],
                                         start=False, stop=True,
                                         skip_group_check=True,
                                         tile_position=(0, 0))
                    jsb = jsbp.tile([128, QG * 128], BF16, tag="jsb")
                    nc.scalar.copy(jsb[:, :], ps[:, :])
                    q0 = blk * QB + half * QG
                    nc.sync.dma_start(
                        out[b, :, q0:q0 + QG, :].rearrange("v j k -> v (j k)"),
                        jsb[:, :],
                    )

            # interleave: emit batch 1's phase right after batch 0's first
            # block so its AT is ready when Vector drains batch 0's SK.
            ctx0 = phase(0)
            ctx1 = phase(1)
            seq = [(ctx0, 0, blk) for blk in range(NQ // QB)]
            seq += [(ctx1, 1, blk) for blk in range(NQ // QB)]
            build_adup(*seq[0])
            for i, (ctx, b, blk) in enumerate(seq):
                nxt = seq[i + 1] if i + 1 < len(seq) else None
                blocks(b, ctx, blk, nxt)

    nc.compile()
    return nc


def _get_nc():
    if "nc" not in _CACHED:
        _CACHED["nc"] = _build()
    return _CACHED["nc"]


def _prep_core_inputs(query, keys, values, i):
    s = slice(i * BPC, (i + 1) * BPC)
    K = np.ascontiguousarray(keys[s])     # (2, 4096, 128) f32
    V = np.ascontiguousarray(values[s])
    Q = np.ascontiguousarray(query[s])    # (2, 64, 128) f32
    kvb = np.empty((BPC, C, 128, 256), dtype=ml_dtypes.bfloat16)
    kvb[:, :, :, 0:128] = V.reshape(BPC, C, 128, 128)
    kvb[:, :, :, 128:256] = K.reshape(BPC, C, 128, 128)
    kt = np.ascontiguousarray(K.transpose(0, 2, 1)).astype(ml_dtypes.bfloat16)
    qt = np.ascontiguousarray(Q.transpose(0, 2, 1)).astype(ml_dtypes.bfloat16)
    return {"kvb": kvb, "kt": kt, "qt": qt}


def kernel(query, keys, values):
    query = np.asarray(query, dtype=np.float32)
    keys = np.asarray(keys, dtype=np.float32)
    values = np.asarray(values, dtype=np.float32)
    nc = _get_nc()
    in_maps = [_prep_core_inputs(query, keys, values, i) for i in range(N_CORES)]
    res = run_bass_kernel_spmd(nc, in_maps, core_ids=list(range(N_CORES)))
    full = np.concatenate(
        [np.asarray(res.results[i]["out"]) for i in range(N_CORES)],
        axis=0)                      # (16, 128v, 64q, 128k)
    return np.ascontiguousarray(full.transpose(0, 2, 1, 3)).astype(np.float32)


# revision 78
# speedup vs baseline: 1.0270x; 1.0017x over previous
"""AttentionJacobian kernel for 8 TRN2 NeuronCores.

J[b,q] = scale * ( V^T diag(a_q) K  -  o_q w_q^T ),  a = softmax(Q K^T scale)

Data-parallel over batch: 16 batches -> 2 per core. Per batch on-device:
  scoresT chunks (n x q) = KT_c^T @ QT        (bf16 matmuls)
  E = exp(scale * scoresT)                    (ScalarE, bf16; no max-sub
                                               needed: |scale*s| small)
  Z = ones^T @ E, rzb = SCALE/Z bcast         (PE + DVE reciprocal)
  AT = E * rzb                                (Pool, bf16, SCALE folded in)
  per 16-q block: ADup = AT pairs [p,c,j,2]   (ScalarE copy)
  SK units (4 chunks x 16 q): sk = K (*) a    as ONE DVE tensor_tensor in
    2x_1P packed mode -- both operands are step-1 bf16 reads (K pairs from
    a dedicated KX copy, a-pairs from ADup), 2 elem/lane/cycle.
    All SK on VectorE: Pool/ACT are 4x/3.5x slower per element and any
    GpSimd load trips the chip power duty-cycler (10.24us k-of-n windows),
    inflating every engine ~2.5x.
  term1 psum += V_c^T @ sk                    (bf16 matmuls, f32 psum,
                                               512-col groups, 8 banks)
  term2: rank-1 PE matmuls psum += (-o_q/SCALE) (x) w_q  (K=1, operands
    are [1,128] rows of SBUF-flattened w/negO at partition 0)
  evacuate psum -> bf16 jsb (ScalarE) -> HBM; host upcasts to f32.
Both batches' softmax phases are emitted before the block loops so the
Vector queue never stalls mid-stream; input DMAs are ordered QT/KT-pieces
first (they gate scores), then KV/KX bulk.
"""

import sys

for p in ("/opt/trn_rl_repo",):
    if p not in sys.path:
        sys.path.append(p)

import numpy as np
import ml_dtypes

import concourse.bass as bass
import concourse.bacc as bacc
import concourse.tile as tile
from concourse import mybir
from concourse.bass_utils import run_bass_kernel_spmd

N_CORES = 8
BATCH = 16
NQ = 64
SEQ = 4096
D = 128
BPC = BATCH // N_CORES        # batches per core = 2
C = SEQ // 128                # 32 contraction chunks
QG = 8                        # q per output group
NG = NQ // QG                 # 8 groups
SCALE = float(D) ** -0.5

F32 = mybir.dt.float32
BF16 = mybir.dt.bfloat16
AF = mybir.ActivationFunctionType
ALU = mybir.AluOpType

_CACHED = {}


def _build():
    nc = bacc.Bacc("TRN2", target_bir_lowering=False, debug=False,
                   num_devices=N_CORES)

    kvb = nc.dram_tensor("kvb", [BPC, C, 128, 256], BF16, kind="ExternalInput").ap()
    kt = nc.dram_tensor("kt", [BPC, 128, SEQ], BF16, kind="ExternalInput").ap()
    qt = nc.dram_tensor("qt", [BPC, 128, NQ], BF16, kind="ExternalInput").ap()
    # [b, v, q, k]: lets each output DMA write one contiguous 2KB block
    # per partition (128 descriptors vs 1024 for the [b,q,v,k] scatter);
    # host transposes to [b, q, v, k] after gather.
    out = nc.dram_tensor("out", [BPC, D, NQ, D], BF16, kind="ExternalOutput").ap()

    with tile.TileContext(nc) as tc:
        with (
            tc.tile_pool(name="const", bufs=1) as constp,
            tc.tile_pool(name="kv", bufs=2) as kvp,
            tc.tile_pool(name="ktp", bufs=2) as ktp,
            tc.tile_pool(name="qtp", bufs=2) as qtp,
            tc.tile_pool(name="ep", bufs=2) as ep,
            tc.tile_pool(name="atp", bufs=2) as atp,
            tc.tile_pool(name="rzp", bufs=2) as rzp,
            tc.tile_pool(name="owp", bufs=2) as owp,
            tc.tile_pool(name="flatp", bufs=2) as flatp,
            tc.tile_pool(name="skp", bufs=4) as skp,
            tc.tile_pool(name="jsbp", bufs=4) as jsbp,
            tc.tile_pool(name="psj", bufs=3, space="PSUM") as psjp,
            tc.tile_pool(name="pss", bufs=1, space="PSUM") as pssp,
            tc.tile_pool(name="psmall", bufs=1, space="PSUM") as psmp,
        ):
            onescol = constp.tile([128, 1], BF16)
            nc.vector.memset(onescol[:, :], 1.0)
            onesrowS = constp.tile([1, 128], F32)
            nc.vector.memset(onesrowS[:, :], SCALE)

            def phase(b):
                # DMA order: QT + KT pieces first (they gate the scores
                # matmuls and everything downstream), then KV/KX bulk.
                QT = qtp.tile([128, NQ], BF16, tag="qt")
                nc.sync.dma_start(QT[:, :], qt[b])
                KT = ktp.tile([128, SEQ], BF16, tag="kt")
                KTPC = 4  # KT DMA pieces
                for piece in range(KTPC):
                    w = SEQ // KTPC
                    nc.sync.dma_start(KT[:, piece * w:(piece + 1) * w],
                                      kt[b, :, piece * w:(piece + 1) * w])

                # --- softmax numerator: E = exp(scale * K Q^T), 4 chunks/exp
                E = ep.tile([128, C * NQ], BF16, tag="e")
                for cc in range(C // 4):
                    ps_s = pssp.tile([128, 4 * NQ], F32, tag="scores")
                    for ci in range(4):
                        c = 4 * cc + ci
                        nc.tensor.matmul(ps_s[:, ci * NQ:(ci + 1) * NQ],
                                         KT[:, c * 128:(c + 1) * 128],
                                         QT[:, :], start=True, stop=True)
                    nc.scalar.activation(E[:, 4 * cc * NQ:(4 * cc + 4) * NQ],
                                         ps_s[:, :], AF.Exp,
                                         bias=0.0, scale=SCALE)

                KV = kvp.tile([128, C * 256], BF16, tag="kv")
                nc.sync.dma_start(KV[:, :].rearrange("p (c j) -> p c j", j=256),
                                  kvb[b].rearrange("c n j -> n c j"))

                # --- Z (1 x NQ) = ones^T E ; rzb = SCALE/Z bcast to 128 parts
                ps_z = psmp.tile([128, 128], F32, tag="small")
                for c in range(C):
                    nc.tensor.matmul(ps_z[0:1, 0:NQ], onescol[:, :],
                                     E[:, c * NQ:(c + 1) * NQ],
                                     start=(c == 0), stop=(c == C - 1))
                rz = rzp.tile([1, NQ], F32, tag="rz")
                nc.vector.reciprocal(rz[:, :], ps_z[0:1, 0:NQ])
                ps_rzb = psmp.tile([128, 128], F32, tag="small")
                nc.tensor.matmul(ps_rzb[:, 0:NQ], onesrowS[:, :], rz[:, :],
                                 start=True, stop=True)

                # --- AT = E * rzb  (chunk-major layout [c*NQ + q])
                rzbsb = rzp.tile([128, NQ], F32, tag="rzbsb")
                nc.scalar.copy(rzbsb[:, :], ps_rzb[:, 0:NQ])
                AT = atp.tile([128, C * NQ], BF16, tag="at")
                H = C // 2
                for hh in range(2):
                    nc.gpsimd.tensor_mul(
                        AT[:, hh * H * NQ:(hh + 1) * H * NQ]
                        .rearrange("p (c q) -> p c q", q=NQ),
                        E[:, hh * H * NQ:(hh + 1) * H * NQ]
                        .rearrange("p (c q) -> p c q", q=NQ),
                        rzbsb[:, :].unsqueeze(1).broadcast_to((128, H, NQ)),
                    )

                # --- w rows [q, k] and o rows [q, v]  (both q-partition)
                ps_ow = psmp.tile([128, 128], F32, tag="small")
                for c in range(C):
                    nc.tensor.matmul(ps_ow[0:NQ, 0:128],
                                     AT[:, c * NQ:(c + 1) * NQ],
                                     KV[:, c * 256 + 128:(c + 1) * 256],
                                     start=(c == 0), stop=(c == C - 1))
                wsb = owp.tile([NQ, 128], BF16, tag="wsb")
                nc.scalar.copy(wsb[:, :], ps_ow[0:NQ, 0:128])
                wflat = flatp.tile([1, NQ * 128], BF16, tag="wflat")
                nc.sync.dma_start(
                    wflat[:, :].rearrange("o (q k) -> o q k", k=128),
                    wsb[:, :])

                ps_o = psmp.tile([128, 128], F32, tag="small")
                for c in range(C):
                    nc.tensor.matmul(ps_o[0:NQ, 0:128],
                                     AT[:, c * NQ:(c + 1) * NQ],
                                     KV[:, c * 256:c * 256 + 128],
                                     start=(c == 0), stop=(c == C - 1))
                negO = owp.tile([NQ, 128], BF16, tag="nego")
                nc.scalar.activation(negO[:, :], ps_o[0:NQ, 0:128],
                                     AF.Copy, bias=0.0, scale=-1.0 / SCALE)
                negOflat = flatp.tile([1, NQ * 128], BF16, tag="negoflat")
                nc.sync.dma_start(
                    negOflat[:, :].rearrange("o (q v) -> o q v", v=128),
                    negO[:, :])
                return dict(KV=KV, KX=KX, AT=AT, wflat=wflat,
                            negOflat=negOflat)

            QB = 2 * QG  # 16 q per block

            adup_tiles = {}

            def build_adup(ctx, b, blk):
                # pair-duplicated AT for a block's 16 q: [p, c, j, 2]
                AT = ctx["AT"]
                ADup = atp.tile([128, C * QB * 2], BF16, tag="adup")
                H = C // 2
                for hh in range(2):
                    nc.scalar.copy(
                        ADup[:, hh * H * QB * 2:(hh + 1) * H * QB * 2]
                        .rearrange("p (c j two) -> p c j two", j=QB, two=2),
                        AT[:, hh * H * NQ:(hh + 1) * H * NQ]
                        .rearrange("p (c q) -> p c q", q=NQ)
                        [:, :, blk * QB:(blk + 1) * QB].unsqueeze(3)
                        .broadcast_to((128, H, QB, 2)),
                    )
                adup_tiles[(b, blk)] = ADup

            def blocks(b, ctx, blk, nxt):
                KV, KX = ctx["KV"], ctx["KX"]
                wflat, negOflat = ctx["wflat"], ctx["negOflat"]
                ADup = adup_tiles.pop((b, blk))
                ps_a = psjp.tile([128, QG * 128], F32, tag="j")
                ps_b = psjp.tile([128, QG * 128], F32, tag="j")
                CU = 4  # chunks per SK unit
                for cc in range(C // CU):
                    c0 = CU * cc
                    sk = skp.tile([128, CU * QB * 128], BF16, tag="sk")
                    # dup-packed 2x_1P: both operands step-1 bf16
                    nc.vector.tensor_mul(
                        sk[:, :].rearrange(
                            "p (ci j kp two) -> p ci j kp two",
                            j=QB, kp=64, two=2),
                        KV[:, :].rearrange(
                            "p (c half kp two) -> p c half kp two",
                            half=2, kp=64, two=2)
                        [:, c0:c0 + CU, 1].unsqueeze(2)
                        .broadcast_to((128, CU, QB, 64, 2)),
                        ADup[:, :].rearrange(
                            "p (c j two) -> p c j two", j=QB, two=2)
                        [:, c0:c0 + CU].unsqueeze(3)
                        .broadcast_to((128, CU, QB, 64, 2)),
                    )
                    for ci in range(CU):
                        c = c0 + ci
                        for h, ps in enumerate((ps_a, ps_a, ps_b, ps_b)):
                            nc.tensor.matmul(
                                ps[:, (h % 2) * 512:(h % 2) * 512 + 512],
                                KV[:, c * 256:c * 256 + 128],
                                sk[:, ci * 2048 + h * 512:
                                   ci * 2048 + (h + 1) * 512],
                                start=(c == 0), stop=False,
                                skip_group_check=True)
                    if cc == 2 and nxt is not None:
                        # prebuild next block's ADup now, ahead of the jsb
                        # copies on the in-order ScalarE queue
                        build_adup(*nxt)
                for half, ps in enumerate((ps_a, ps_b)):
                    for j in range(QG):
                        q = blk * QB + half * QG + j
                        nc.tensor.matmul(ps[:, j * 128:(j + 1) * 128],
                                         negOflat[:, q * 128:(q + 1) * 128],
                                         wflat[:, q * 128:(q + 1) * 128],
                                         start=False, stop=True,
                                         skip_group_check=True,
                                         tile_position=(0, 0))
                    jsb = jsbp.tile([128, QG * 128], BF16, tag="jsb")
                    nc.scalar.copy(jsb[:, :], ps[:, :])
                    q0 = blk * QB + half * QG
                    nc.sync.dma_start(
                        out[b, :, q0:q0 + QG, :].rearrange("v j k -> v (j k)"),
                        jsb[:, :],
                    )

            # interleave: emit batch 1's phase right after batch 0's first
            # block so its AT is ready when Vector drains batch 0's SK.
            ctx0 = phase(0)
            ctx1 = phase(1)
            seq = [(ctx0, 0, blk) for blk in range(NQ // QB)]
            seq += [(ctx1, 1, blk) for blk in range(NQ // QB)]
            build_adup(*seq[0])
            for i, (ctx, b, blk) in enumerate(seq):
                nxt = seq[i + 1] if i + 1 < len(seq) else None
                blocks(b, ctx, blk, nxt)

    nc.compile()
    return nc


def _get_nc():
    if "nc" not in _CACHED:
        _CACHED["nc"] = _build()
    return _CACHED["nc"]


def _prep_core_inputs(query, keys, values, i):
    s = slice(i * BPC, (i + 1) * BPC)
    K = np.ascontiguousarray(keys[s])     # (2, 4096, 128) f32
    V = np.ascontiguousarray(values[s])
    Q = np.ascontiguousarray(query[s])    # (2, 64, 128) f32
    kvb = np.empty((BPC, C, 128, 256), dtype=ml_dtypes.bfloat16)
    kvb[:, :, :, 0:128] = V.reshape(BPC, C, 128, 128)
    kvb[:, :, :, 128:256] = K.reshape(BPC, C, 128, 128)
    kt = np.ascontiguousarray(K.transpose(0, 2, 1)).astype(ml_dtypes.bfloat16)
    qt = np.ascontiguousarray(Q.transpose(0, 2, 1)).astype(ml_dtypes.bfloat16)
    return {"kvb": kvb, "kt": kt, "qt": qt}


def kernel(query, keys, values):
    query = np.asarray(query, dtype=np.float32)
    keys = np.asarray(keys, dtype=np.float32)
    values = np.asarray(values, dtype=np.float32)
    nc = _get_nc()
    in_maps = [_prep_core_inputs(query, keys, values, i) for i in range(N_CORES)]
    res = run_bass_kernel_spmd(nc, in_maps, core_ids=list(range(N_CORES)))
    full = np.concatenate(
        [np.asarray(res.results[i]["out"]) for i in range(N_CORES)],
        axis=0)                      # (16, 128v, 64q, 128k)
    return np.ascontiguousarray(full.transpose(0, 2, 1, 3)).astype(np.float32)
